# revision 16
# baseline (speedup 1.0000x reference)
"""Multi-head self-attention (pre-LN, residual) Trainium2 Bass kernel, v2.

Problem: B=4, S=2048, D=128, H=4, Dh=32, fp32.
Sharding: 8 cores = 4 batches x 2 query-halves (1024 queries/core).
Each core receives its batch's full x, row-shuffled by the host so that
(a) the core's query half occupies device positions 0..1023 and (b) each
SBUF partition loads consecutive DRAM rows.

Dataflow ([feature, seq] layouts), per core:
  xn0^T --W--> Q^T,K^T [hd, s] bf16;  V [s, hd] bf16 with a per-head
                                      ones column appended (33 cols/head)
  S^T[k,q] = K^T.T @ Q^T   4 heads row-tiled (K=32 at rows h*32)
  P_A = exp(S^T - 8)       heads {0,2} on ACT (one [128,1024] op)
  P_B = schraudolph(S^T-8) heads {1,3} on DVE (per-head [128,512] ops:
                           int16(x*SA+SB) bits == bf16 exp)
  ctx+den fused: M=33 col-tiled matmuls, bankA={h0@0,h2@64},
                 bankB={h1@0,h3@64}; row 32/96 of each bank = den
  deninv: K=1 ones-matmul broadcasts den rows to [128,512], DVE fast
          reciprocal, multiply, then 4 K=32 row-positioned Wo matmuls
          accumulate the output projection (junk rows never read)
  out^T = Wo.T @ ctxn + (x^T + rbias)

Host folds gamma/beta/biases/ISQ into the projection weights (numpy),
permutes Wo rows to the 2-bank ctx layout, and ships x^T for the
residual, so the device does no weight prep.  LN rsqrt runs on DVE
(quake seed + 2 Newton steps) so ACT keeps one table set (exp) loaded.

PSUM (8 banks): sA scores [128,1024] x2 bufs = 4, sB0/sB1 per-head
[128,512] = 2, ctxA/ctxB = 2.  Prep and tail psum tiles ride the sA
ring (2-buf rotation absorbs single insertions); prep is dripped one
psum tile per attention iteration; chunk-0's tail overlaps chunk-1.
"""

import sys

if "/opt/trn_rl_repo" not in sys.path:
    sys.path.insert(0, "/opt/trn_rl_repo")

import numpy as np

import concourse.bacc as bacc
import concourse.tile as tile
import concourse.mybir as mybir
from concourse.bass_utils import run_bass_kernel_spmd
from concourse.masks import make_identity

F32 = mybir.dt.float32
F32R = mybir.dt.float32r
BF16 = mybir.dt.bfloat16
I16 = mybir.dt.int16
I32 = mybir.dt.int32
AF = mybir.ActivationFunctionType
OP = mybir.AluOpType

B, S, D = 4, 2048, 128
H, DH = 4, 32
N_CORES = 8
QH = S // 2  # queries per core
NT = S // 128  # 16 s-tiles
CHUNK = 512
NCH = QH // CHUNK  # q-chunks per core (2)
NKT = S // 128  # 16 k-tiles
EPS = 1e-6
SHIFT = 8.0
ISQ = 1.0 / np.sqrt(np.float32(DH))
# Schraudolph bf16 exp: int16(x*SA + SB).bits == bf16(exp(x - SHIFT))
SA = float(128.0 / np.log(2.0))
SB = float(127.0 * 128.0 - 0.0579 * 128.0 - SHIFT * 128.0 / np.log(2.0))
QK3 = 0x5F3759DF  # quake rsqrt seed

_compiled = None


def _build():
    nc = bacc.Bacc(
        "TRN2",
        target_bir_lowering=False,
        debug=False,
        enable_asserts=False,
        num_devices=N_CORES,
    )

    xkv_d = nc.dram_tensor("xkv", [S, D], F32, kind="ExternalInput").ap()
    xt_d = nc.dram_tensor("xt", [D, QH], F32, kind="ExternalInput").ap()
    wq_d = nc.dram_tensor("wq", [D, D], BF16, kind="ExternalInput").ap()
    wk_d = nc.dram_tensor("wk", [D, D], BF16, kind="ExternalInput").ap()
    wv_d = nc.dram_tensor("wv", [D, D], BF16, kind="ExternalInput").ap()
    # woAB[0] rows {0-31: h0, 64-95: h2}; woAB[1] rows {0-31: h1, 64-95: h3}
    woAB_d = nc.dram_tensor("woAB", [2, D, D], F32R, kind="ExternalInput").ap()
    # rows: bq_eff, bk_eff, rbias
    vecs_d = nc.dram_tensor("vecs", [3, D], F32, kind="ExternalInput").ap()
    outT_d = nc.dram_tensor("outT", [D, QH], F32, kind="ExternalOutput").ap()

    with tile.TileContext(nc) as tc:
        consts = tc.alloc_tile_pool(name="consts", bufs=1)
        sbW = tc.alloc_tile_pool(name="sbW", bufs=1)
        sbBig = tc.alloc_tile_pool(name="sbBig", bufs=1)
        sbTmp = tc.alloc_tile_pool(name="sbTmp", bufs=3)

        ident = consts.tile([128, 128], F32)
        make_identity(nc, ident)
        nshift = consts.tile([128, 1], F32)
        nc.vector.memset(nshift, -SHIFT)
        wsrc = consts.tile([128, 512], BF16)
        nc.vector.memset(wsrc, 0.5)
        wones = consts.tile([128, DH], BF16)
        nc.vector.memset(wones, 1.0)
        msel = consts.tile([128, 128], F32)
        nc.vector.memset(msel, 0.0)
        nc.vector.memset(msel[0:1, 0:64], 1.0)
        nc.vector.memset(msel[64:65, 64:128], 1.0)
        dummy = consts.tile([128, 1], F32)
        nc.vector.memset(dummy, 0.0)

        # ---- input DMAs ----
        wq_f = sbW.tile([D, D], BF16)
        wk_f = sbW.tile([D, D], BF16)
        wv_f = sbW.tile([D, D], BF16)
        wo_sb = sbW.tile([D, 2, D], F32R)
        nc.scalar.dma_start(out=wq_f, in_=wq_d)
        nc.scalar.dma_start(out=wk_f, in_=wk_d)
        nc.scalar.dma_start(out=wv_f, in_=wv_d)
        nc.scalar.dma_start(out=wo_sb, in_=woAB_d.rearrange("g d e -> d g e"))
        vecsT = sbW.tile([D, 3], F32)  # cols: bq_eff, bk_eff, rbias
        nc.scalar.dma_start(out=vecsT, in_=vecs_d.rearrange("v d -> d v"))
        residT = sbBig.tile([128, QH], F32)  # x^T + rbias (query half)
        xt_sb = sbBig.tile([128, QH], F32)

        xkv_sb = sbBig.tile([128, NT, 128], F32)
        xkv_r = xkv_d.rearrange("(p t) d -> p t d", t=NT)
        for c4 in range(4):
            nc.sync.dma_start(
                out=xkv_sb[:, c4 * 4 : (c4 + 1) * 4, :],
                in_=xkv_r[:, c4 * 4 : (c4 + 1) * 4, :],
            )
        nc.gpsimd.dma_start(out=xt_sb, in_=xt_d)

        # ---- PSUM pool: sA 2x[128,1024]=4 banks, sB0/sB1/ctxA/ctxB 1 each ----
        ps = tc.alloc_tile_pool(name="ps", bufs=1, space="PSUM")

        def sA_tile(name):
            return ps.tile([128, 2 * CHUNK], F32, name=name, tag="sA", bufs=2)

        # force the exp table load early (hides the ~1.3us load in startup)
        warm_exp = sbTmp.tile([128, 1], F32, tag="we")
        nc.scalar.activation(warm_exp, dummy, AF.Exp, bias=nshift, scale=1.0)

        # HAM warm-up chain on the ctx banks (no consumers -> back-to-back)
        for _ in range(8):
            wps = ps.tile([128, CHUNK], F32, name="wps", tag="ctxA", bufs=1)
            nc.tensor.matmul(wps[0:DH, :], wones, wsrc, start=True, stop=True)

        bqe = vecsT[:, 0:1]
        bke = vecsT[:, 1:2]
        rbias = vecsT[:, 2:3]


        # ---- LayerNorm + transposes + projections ----
        mv_all = sbBig.tile([128, NT, 2], F32)
        rs_all = sbBig.tile([128, NT], F32)
        q1 = sbBig.tile([128, NT], F32)
        q2 = sbBig.tile([128, NT], F32)
        xn0_sb = sbBig.tile([128, NT, 128], F32)
        xkvT = sbBig.tile([128, S], BF16)  # xn0^T [d, s]
        kT = sbBig.tile([128, S], BF16)
        qT = sbBig.tile([128, QH], BF16)
        # V per head: 64 cols = [ones (den), 32 v-dims, 31 zeros]
        v_sb = sbBig.tile([128, NT, H, 64], BF16)
        nc.vector.memset(v_sb[:, :, :, 33:64], 0.0)
        nc.vector.memset(v_sb[:, :, :, 0:1], 1.0)

        def quake_rsqrt(sl4):
            # rs = 1/sqrt(var+eps), all on DVE (avoids ACT Sqrt table swap)
            va = mv_all[:, sl4, 1]
            a = rs_all[:, sl4]
            nc.vector.tensor_scalar_add(a, va, float(EPS))
            u = a.bitcast(I32)
            y = q1[:, sl4]
            yi = y.bitcast(I32)
            nc.vector.tensor_scalar(
                yi, u, 1, 0, op0=OP.logical_shift_right, op1=OP.bypass
            )
            nc.vector.tensor_scalar(yi, yi, -1, QK3, op0=OP.mult, op1=OP.add)
            t = q2[:, sl4]
            for it in range(2):
                # y = y * (1.5 - 0.5*a*y*y)
                nc.vector.tensor_mul(t, y, y)
                nc.vector.tensor_mul(t, t, a)
                nc.vector.tensor_scalar(t, t, -0.5, 1.5, op0=OP.mult, op1=OP.add)
                if it == 0:
                    nc.vector.tensor_mul(y, y, t)
                else:
                    nc.vector.tensor_mul(rs_all[:, sl4], y, t)

        def prep_ln(b4):
            # DVE-only part of a prep block (no psum)
            for t in range(b4 * 4, b4 * 4 + 4):
                stats = sbTmp.tile([128, 6], F32, tag="st")
                nc.vector.bn_stats(stats, xkv_sb[:, t, :])
                nc.vector.bn_aggr(mv_all[:, t, :], stats)
            sl4 = slice(b4 * 4, b4 * 4 + 4)
            quake_rsqrt(sl4)
            for t in range(b4 * 4, b4 * 4 + 4):
                nc.gpsimd.tensor_scalar(
                    xn0_sb[:, t, :],
                    xkv_sb[:, t, :],
                    mv_all[:, t, 0:1],
                    rs_all[:, t : t + 1],
                    op0=OP.subtract,
                    op1=OP.mult,
                )

        def prep_tp(b4):
            # 4 transposes into one sA tile, one ACT copy -> xkvT (bf16)
            tp = sA_tile("tp")
            for i, t in enumerate(range(b4 * 4, b4 * 4 + 4)):
                nc.tensor.transpose(
                    tp[:, i * 128 : (i + 1) * 128], xn0_sb[:, t, :], ident
                )
            nc.scalar.copy(
                xkvT[:, b4 * 512 : (b4 + 1) * 512], tp[:, 0:512]
            )

        def prep_kq(b4):
            c = b4
            pp = sA_tile("pp")
            nc.tensor.matmul(
                pp[:, 0:CHUNK], wk_f, xkvT[:, c * CHUNK : (c + 1) * CHUNK],
                start=True, stop=True,
            )
            nc.scalar.add(kT[:, c * CHUNK : (c + 1) * CHUNK], pp[:, 0:CHUNK], bke)
            if c < NCH:
                nc.tensor.matmul(
                    pp[:, CHUNK : 2 * CHUNK],
                    wq_f,
                    xkvT[:, c * CHUNK : (c + 1) * CHUNK],
                    start=True,
                    stop=True,
                )
                nc.scalar.add(
                    qT[:, c * CHUNK : (c + 1) * CHUNK], pp[:, CHUNK : 2 * CHUNK], bqe
                )

        def prep_v(b4):
            # 4 v-proj matmuls into one sA tile, one strided ACT copy
            vp = sA_tile("vp")
            for i, t in enumerate(range(b4 * 4, b4 * 4 + 4)):
                nc.tensor.matmul(
                    vp[:, i * 128 : (i + 1) * 128],
                    xkvT[:, t * 128 : (t + 1) * 128],
                    wv_f,
                    start=True,
                    stop=True,
                )
            sl4 = slice(b4 * 4, b4 * 4 + 4)
            vpv = vp[:, 0:512].rearrange("p (t h d) -> p t h d", t=4, h=4, d=32)
            nc.scalar.copy(v_sb[:, sl4, :, 1:33], vpv)

        # ---- attention ----
        pPool = tc.alloc_tile_pool(name="pPool", bufs=3)

        ctx_ps = {}

        def attn_scores(qc, kt):
            q0 = qc * CHUNK
            k0 = kt * 128
            # group A: heads 0,2 -> one [128,1024] psum tile, ACT exp
            sa = sA_tile("sa")
            for i, h in enumerate((0, 2)):
                nc.tensor.matmul(
                    sa[:, i * CHUNK : (i + 1) * CHUNK],
                    kT[h * DH : (h + 1) * DH, k0 : k0 + 128],
                    qT[h * DH : (h + 1) * DH, q0 : q0 + CHUNK],
                    start=True,
                    stop=True,
                    tile_position=(h * DH, 0),
                )
            pA = pPool.tile([128, 2 * CHUNK], BF16, tag="pA")
            nc.scalar.activation(pA, sa, AF.Exp, bias=nshift, scale=1.0)
            # group B: heads 1,3 -> per-head [128,512] psum tiles, DVE exp
            pBs = []
            for i, h in enumerate((1, 3)):
                sb = ps.tile(
                    [128, CHUNK], F32, name=f"sb{i}", tag=f"sB{i}", bufs=1
                )
                nc.tensor.matmul(
                    sb,
                    kT[h * DH : (h + 1) * DH, k0 : k0 + 128],
                    qT[h * DH : (h + 1) * DH, q0 : q0 + CHUNK],
                    start=True,
                    stop=True,
                    tile_position=(h * DH, 0),
                )
                pB = pPool.tile([128, CHUNK], I16, tag=f"pB{i}")
                nc.vector.tensor_scalar(pB, sb, SA, SB, op0=OP.mult, op1=OP.add)
                pBs.append(pB.bitcast(BF16))
            return pA, pBs

        def attn_ctx(kt, p_sb):
            pA, pBs = p_sb
            first, last = kt == 0, kt == NKT - 1
            # M=64 col-tiled: h at rows 64i..64i+31, row 64i+32 = den,
            # rows 64i+33..64i+63 = 0 (keeps the whole bank initialized)
            for i, h in enumerate((0, 2)):
                nc.tensor.matmul(
                    ctx_ps["A"][64 * i : 64 * i + 64, :],
                    v_sb[:, kt, h, :],
                    pA[:, i * CHUNK : (i + 1) * CHUNK],
                    start=first,
                    stop=last,
                    tile_position=(0, 64 * i),
                    skip_group_check=True,
                )
            for i, h in enumerate((1, 3)):
                nc.tensor.matmul(
                    ctx_ps["B"][64 * i : 64 * i + 64, :],
                    v_sb[:, kt, h, :],
                    pBs[i],
                    start=first,
                    stop=last,
                    tile_position=(0, 64 * i),
                    skip_group_check=True,
                )

        tail_state = {}

        def tail_copy(qc, cps):
            # psum->sbuf unload of both ctx banks (frees the banks)
            st = {}
            for g in ("A", "B"):
                cs = sbTmp.tile([128, CHUNK], F32, tag=f"cs{g}")
                nc.scalar.copy(cs, cps[g])
                st[g] = cs
            tail_state[qc] = st

        def tail_div(qc, g):
            # den broadcast via masked fp32 matmul, fast recip, multiply
            cs = tail_state[qc][g]
            dps = sA_tile(f"dps{g}")
            nc.tensor.matmul(dps[:, 0:CHUNK], msel, cs, start=True, stop=True)
            dinv = sbTmp.tile([128, CHUNK], F32, tag=f"di{g}")
            nc.vector.reciprocal_approx_fast(dinv, dps[:, 0:CHUNK])
            ctxn = sbTmp.tile([128, CHUNK], F32R, tag=f"cn{g}")
            nc.vector.tensor_mul(ctxn, cs, dinv)
            tail_state[qc][g + "n"] = ctxn

        def tail_out(qc):
            q0 = qc * CHUNK
            outp = sA_tile("outp")
            for gi, g in enumerate(("A", "B")):
                ctxn = tail_state[qc][g + "n"]
                nc.tensor.matmul(
                    outp[:, 0:CHUNK],
                    wo_sb[:, gi, :],
                    ctxn,
                    start=(gi == 0),
                    stop=(gi == 1),
                )
            fin = sbTmp.tile([128, CHUNK], F32, tag="fin")
            nc.vector.tensor_add(fin, outp[:, 0:CHUNK], residT[:, q0 : q0 + CHUNK])
            nc.sync.dma_start(out=outT_d[:, q0 : q0 + CHUNK], in_=fin)

        # ---- schedule ----
        prep_ln(0)
        prep_tp(0)
        prep_kq(0)
        prep_v(0)
        prep_ln(1)
        prep_tp(1)
        prep_kq(1)
        prep_v(1)
        # residT = xt + rbias (gpsimd: off the DVE critical path)
        nc.gpsimd.tensor_scalar_add(residT[:, 0:CHUNK], xt_sb[:, 0:CHUNK], rbias)
        nc.gpsimd.tensor_scalar_add(
            residT[:, CHUNK:QH], xt_sb[:, CHUNK:QH], rbias
        )

        # chunk 0; prep blocks 2,3 dripped one psum tile per iteration
        ctx_ps = {
            "A": ps.tile([128, CHUNK], F32, name="ctxA0", tag="ctxA", bufs=1),
            "B": ps.tile([128, CHUNK], F32, name="ctxB0", tag="ctxB", bufs=1),
        }
        drip = [
            lambda: prep_ln(2),
            lambda: prep_tp(2),
            lambda: prep_kq(2),
            lambda: prep_v(2),
            lambda: prep_ln(3),
            lambda: prep_tp(3),
            lambda: prep_kq(3),
            lambda: prep_v(3),
        ]
        pending = attn_scores(0, 0)
        for kt in range(NKT):
            if kt >= 1 and drip:
                drip.pop(0)()
            nxt = attn_scores(0, kt + 1) if kt + 1 < NKT else None
            attn_ctx(kt, pending)
            pending = nxt

        ctx0 = ctx_ps
        tail_copy(0, ctx0)

        # chunk 1; chunk-0 tail pieces interleaved
        ctx_ps = {
            "A": ps.tile([128, CHUNK], F32, name="ctxA1", tag="ctxA", bufs=1),
            "B": ps.tile([128, CHUNK], F32, name="ctxB1", tag="ctxB", bufs=1),
        }
        pending = attn_scores(1, 0)
        for kt in range(NKT):
            if kt == 2:
                tail_div(0, "A")
            elif kt == 4:
                tail_div(0, "B")
            elif kt == 6:
                tail_out(0)
            nxt = attn_scores(1, kt + 1) if kt + 1 < NKT else None
            attn_ctx(kt, pending)
            pending = nxt

        # chunk-1 endgame: half-width pipeline so the first out-DMA starts early
        q0 = CHUNK
        csA = sbTmp.tile([128, CHUNK], F32, tag="csA")
        csB = sbTmp.tile([128, CHUNK], F32, tag="csB")
        diA = sbTmp.tile([128, CHUNK], F32, tag="diA")
        diB = sbTmp.tile([128, CHUNK], F32, tag="diB")
        cnA = sbTmp.tile([128, CHUNK], F32R, tag="cnA")
        cnB = sbTmp.tile([128, CHUNK], F32R, tag="cnB")
        fin = sbTmp.tile([128, CHUNK], F32, tag="fin")
        dps = sA_tile("dpsf")
        outp = sA_tile("outpf")
        for hf in range(2):
            sl = slice(hf * 256, (hf + 1) * 256)
            nc.scalar.copy(csA[:, sl], ctx_ps["A"][:, sl])
            nc.scalar.copy(csB[:, sl], ctx_ps["B"][:, sl])
            d0 = hf * 512
            nc.tensor.matmul(
                dps[:, d0 : d0 + 256], msel, csA[:, sl], start=True, stop=True
            )
            nc.tensor.matmul(
                dps[:, d0 + 256 : d0 + 512], msel, csB[:, sl], start=True, stop=True
            )
            nc.vector.reciprocal_approx_fast(diA[:, sl], dps[:, d0 : d0 + 256])
            nc.vector.reciprocal_approx_fast(diB[:, sl], dps[:, d0 + 256 : d0 + 512])
            nc.vector.tensor_mul(cnA[:, sl], csA[:, sl], diA[:, sl])
            nc.vector.tensor_mul(cnB[:, sl], csB[:, sl], diB[:, sl])
            o0 = hf * 256
            nc.tensor.matmul(
                outp[:, o0 : o0 + 256], wo_sb[:, 0, :], cnA[:, sl],
                start=True, stop=False,
            )
            nc.tensor.matmul(
                outp[:, o0 : o0 + 256], wo_sb[:, 1, :], cnB[:, sl],
                start=False, stop=True,
            )
            nc.vector.tensor_add(
                fin[:, sl], outp[:, o0 : o0 + 256], residT[:, q0 + hf * 256 : q0 + (hf + 1) * 256]
            )
            nc.sync.dma_start(
                out=outT_d[:, q0 + hf * 256 : q0 + (hf + 1) * 256], in_=fin[:, sl]
            )

        pPool.release()
        ps.release()
        sbTmp.release()
        sbBig.release()
        sbW.release()
        consts.release()

    nc.compile()
    return nc


def _get_compiled():
    global _compiled
    if _compiled is None:
        _compiled = _build()
    return _compiled


# device position j <- host row (j%128)*16 + j//128
_DEV2HOST = (np.arange(S) % 128) * NT + np.arange(S) // 128
_HOSTPERM = np.empty(S, dtype=np.int64)
_HOSTPERM[_DEV2HOST] = np.arange(S)


def kernel(x, Wq, bq, Wk, bk, Wv, bv, gamma, beta, Wo, bo):
    bf16 = mybir.dt.np(BF16)
    x = np.asarray(x, dtype=np.float32)
    Wq = np.asarray(Wq, dtype=np.float64)
    Wk = np.asarray(Wk, dtype=np.float64)
    Wv = np.asarray(Wv, dtype=np.float64)
    Wo = np.asarray(Wo, dtype=np.float64)
    gamma = np.asarray(gamma, dtype=np.float64)
    beta = np.asarray(beta, dtype=np.float64)
    bq = np.asarray(bq, dtype=np.float64)
    bk = np.asarray(bk, dtype=np.float64)
    bv = np.asarray(bv, dtype=np.float64)
    bo = np.asarray(bo, dtype=np.float64)

    # fold gamma (and ISQ into q) into the projections; beta into biases
    wq_f = np.ascontiguousarray((Wq * gamma[:, None] * ISQ).astype(bf16))
    wk_f = np.ascontiguousarray((Wk * gamma[:, None]).astype(bf16))
    wv_f = np.ascontiguousarray((Wv * gamma[:, None]).astype(bf16))
    bq_eff = (Wq.T @ beta + bq) * ISQ
    bk_eff = Wk.T @ beta + bk
    bv_eff = Wv.T @ beta + bv
    rbias = Wo.T @ bv_eff + bo

    # Wo rows permuted to the 2-bank ctx layout:
    # bank A holds h0 at partitions 0-31, h2 at 64-95; bank B h1/h3.
    woAB = np.zeros((2, D, D), dtype=np.float64)
    woAB[0, 1:33] = Wo[0 * DH : 1 * DH]
    woAB[0, 65:97] = Wo[2 * DH : 3 * DH]
    woAB[1, 1:33] = Wo[1 * DH : 2 * DH]
    woAB[1, 65:97] = Wo[3 * DH : 4 * DH]
    woAB = np.ascontiguousarray(woAB.astype(np.float32))

    vecs = np.ascontiguousarray(
        np.stack([bq_eff, bk_eff, rbias]).astype(np.float32)
    )

    nc = _get_compiled()

    in_maps = []
    for c in range(N_CORES):
        b, half = c // 2, c % 2
        off = half * QH
        xroll = np.roll(x[b], -off, axis=0)
        xin = np.ascontiguousarray(xroll[_HOSTPERM])
        xt = np.ascontiguousarray(xroll[0:QH].T)
        in_maps.append(
            {
                "xkv": xin,
                "xt": xt,
                "wq": wq_f,
                "wk": wk_f,
                "wv": wv_f,
                "woAB": woAB,
                "vecs": vecs,
            }
        )

    res = run_bass_kernel_spmd(nc, in_maps, core_ids=list(range(N_CORES)), trace=False)

    out = np.empty((B, S, D), dtype=np.float32)
    for c in range(N_CORES):
        b, half = c // 2, c % 2
        off = half * QH
        out[b, off : off + QH, :] = res.results[c]["outT"].T
    return out


# revision 17
# speedup vs baseline: 1.2491x; 1.2491x over previous
"""Multi-head self-attention (pre-LN, residual) Trainium2 Bass kernel, v2.

Problem: B=4, S=2048, D=128, H=4, Dh=32, fp32.
Sharding: 8 cores = 4 batches x 2 query-halves (1024 queries/core).
Each core receives its batch's full x, row-shuffled by the host so that
(a) the core's query half occupies device positions 0..1023 and (b) each
SBUF partition loads consecutive DRAM rows.

Dataflow ([feature, seq] layouts), per core:
  xn0^T --W--> Q^T,K^T [hd, s] bf16;  V [s, hd] bf16 with a per-head
                                      ones column appended (33 cols/head)
  S^T[k,q] = K^T.T @ Q^T   4 heads row-tiled (K=32 at rows h*32)
  P_A = exp(S^T - 8)       heads {0,2} on ACT (one [128,1024] op)
  P_B = schraudolph(S^T-8) heads {1,3} on DVE (per-head [128,512] ops:
                           int16(x*SA+SB) bits == bf16 exp)
  ctx+den fused: M=33 col-tiled matmuls, bankA={h0@0,h2@64},
                 bankB={h1@0,h3@64}; row 32/96 of each bank = den
  deninv: K=1 ones-matmul broadcasts den rows to [128,512], DVE fast
          reciprocal, multiply, then 4 K=32 row-positioned Wo matmuls
          accumulate the output projection (junk rows never read)
  out^T = Wo.T @ ctxn + (x^T + rbias)

Host folds gamma/beta/biases/ISQ into the projection weights (numpy),
permutes Wo rows to the 2-bank ctx layout, and ships x^T for the
residual, so the device does no weight prep.  LN rsqrt runs on DVE
(quake seed + 2 Newton steps) so ACT keeps one table set (exp) loaded.

PSUM (8 banks): sA scores [128,1024] x2 bufs = 4, sB0/sB1 per-head
[128,512] = 2, ctxA/ctxB = 2.  Prep and tail psum tiles ride the sA
ring (2-buf rotation absorbs single insertions); prep is dripped one
psum tile per attention iteration; chunk-0's tail overlaps chunk-1.
"""

import sys

if "/opt/trn_rl_repo" not in sys.path:
    sys.path.insert(0, "/opt/trn_rl_repo")

import numpy as np

import concourse.bacc as bacc
import concourse.tile as tile
import concourse.mybir as mybir
from concourse.bass_utils import run_bass_kernel_spmd
from concourse.masks import make_identity

F32 = mybir.dt.float32
F32R = mybir.dt.float32r
BF16 = mybir.dt.bfloat16
I16 = mybir.dt.int16
I32 = mybir.dt.int32
AF = mybir.ActivationFunctionType
OP = mybir.AluOpType

B, S, D = 4, 2048, 128
H, DH = 4, 32
N_CORES = 8
QH = S // 2  # queries per core
NT = S // 128  # 16 s-tiles
CHUNK = 512
NCH = QH // CHUNK  # q-chunks per core (2)
NKT = S // 128  # 16 k-tiles
EPS = 1e-6
SHIFT = 8.0
ISQ = 1.0 / np.sqrt(np.float32(DH))
# Schraudolph bf16 exp: int16(x*SA + SB).bits == bf16(exp(x - SHIFT))
SA = float(128.0 / np.log(2.0))
SB = float(127.0 * 128.0 - 0.0579 * 128.0 - SHIFT * 128.0 / np.log(2.0))
QK3 = 0x5F3759DF  # quake rsqrt seed

_compiled = None


def _build():
    nc = bacc.Bacc(
        "TRN2",
        target_bir_lowering=False,
        debug=False,
        enable_asserts=False,
        num_devices=N_CORES,
    )

    xkv_d = nc.dram_tensor("xkv", [S, D], F32, kind="ExternalInput").ap()
    xt_d = nc.dram_tensor("xt", [D, QH], F32, kind="ExternalInput").ap()
    wq_d = nc.dram_tensor("wq", [D, D], BF16, kind="ExternalInput").ap()
    wk_d = nc.dram_tensor("wk", [D, D], BF16, kind="ExternalInput").ap()
    wv_d = nc.dram_tensor("wv", [D, D], BF16, kind="ExternalInput").ap()
    # woAB[0] rows {0-31: h0, 64-95: h2}; woAB[1] rows {0-31: h1, 64-95: h3}
    woAB_d = nc.dram_tensor("woAB", [2, D, D], F32R, kind="ExternalInput").ap()
    # rows: bq_eff, bk_eff, rbias
    vecs_d = nc.dram_tensor("vecs", [3, D], F32, kind="ExternalInput").ap()
    outT_d = nc.dram_tensor("outT", [D, QH], F32, kind="ExternalOutput").ap()

    with tile.TileContext(nc) as tc:
        consts = tc.alloc_tile_pool(name="consts", bufs=1)
        sbW = tc.alloc_tile_pool(name="sbW", bufs=1)
        sbBig = tc.alloc_tile_pool(name="sbBig", bufs=1)
        sbTmp = tc.alloc_tile_pool(name="sbTmp", bufs=3)

        ident = consts.tile([128, 128], F32)
        make_identity(nc, ident)
        nshift = consts.tile([128, 1], F32)
        nc.vector.memset(nshift, -SHIFT)
        wsrc = consts.tile([128, 512], BF16)
        nc.vector.memset(wsrc, 0.5)
        wones = consts.tile([128, DH], BF16)
        nc.vector.memset(wones, 1.0)
        msel = consts.tile([128, 128], F32)
        nc.vector.memset(msel, 0.0)
        nc.vector.memset(msel[0:1, 0:64], 1.0)
        nc.vector.memset(msel[64:65, 64:128], 1.0)
        dummy = consts.tile([128, 1], F32)
        nc.vector.memset(dummy, 0.0)

        # ---- input DMAs ----
        wq_f = sbW.tile([D, D], BF16)
        wk_f = sbW.tile([D, D], BF16)
        wv_f = sbW.tile([D, D], BF16)
        wo_sb = sbW.tile([D, 2, D], F32R)
        nc.scalar.dma_start(out=wq_f, in_=wq_d)
        nc.scalar.dma_start(out=wk_f, in_=wk_d)
        nc.scalar.dma_start(out=wv_f, in_=wv_d)
        nc.scalar.dma_start(out=wo_sb, in_=woAB_d.rearrange("g d e -> d g e"))
        vecsT = sbW.tile([D, 3], F32)  # cols: bq_eff, bk_eff, rbias
        nc.scalar.dma_start(out=vecsT, in_=vecs_d.rearrange("v d -> d v"))
        residT = sbBig.tile([128, QH], F32)  # x^T + rbias (query half)
        xt_sb = sbBig.tile([128, QH], F32)

        xkv_sb = sbBig.tile([128, NT, 128], F32)
        xkv_r = xkv_d.rearrange("(p t) d -> p t d", t=NT)
        for c4 in range(4):
            nc.sync.dma_start(
                out=xkv_sb[:, c4 * 4 : (c4 + 1) * 4, :],
                in_=xkv_r[:, c4 * 4 : (c4 + 1) * 4, :],
            )
        nc.gpsimd.dma_start(out=xt_sb, in_=xt_d)

        # ---- PSUM pool: sA 2x[128,1024]=4 banks, sB0/sB1/ctxA/ctxB 1 each ----
        ps = tc.alloc_tile_pool(name="ps", bufs=1, space="PSUM")

        def sA_tile(name):
            return ps.tile([128, 2 * CHUNK], F32, name=name, tag="sA", bufs=2)

        # force the exp table load early (hides the ~1.3us load in startup)
        warm_exp = sbTmp.tile([128, 1], F32, tag="we")
        nc.scalar.activation(warm_exp, dummy, AF.Exp, bias=nshift, scale=1.0)

        # HAM warm-up chain on the ctx banks (no consumers -> back-to-back)
        for _ in range(8):
            wps = ps.tile([128, CHUNK], F32, name="wps", tag="ctxA", bufs=1)
            nc.tensor.matmul(wps[0:DH, :], wones, wsrc, start=True, stop=True)

        bqe = vecsT[:, 0:1]
        bke = vecsT[:, 1:2]
        rbias = vecsT[:, 2:3]


        # ---- LayerNorm + transposes + projections ----
        mv_all = sbBig.tile([128, NT, 2], F32)
        rs_all = sbBig.tile([128, NT], F32)
        q1 = sbBig.tile([128, NT], F32)
        q2 = sbBig.tile([128, NT], F32)
        xn0_sb = sbBig.tile([128, NT, 128], F32)
        xkvT = sbBig.tile([128, S], BF16)  # xn0^T [d, s]
        kT = sbBig.tile([128, S], BF16)
        qT = sbBig.tile([128, QH], BF16)
        # V per head: 64 cols = [ones (den), 32 v-dims, 31 zeros]
        v_sb = sbBig.tile([128, NT, H, 64], BF16)
        nc.vector.memset(v_sb[:, :, :, 33:64], 0.0)
        nc.vector.memset(v_sb[:, :, :, 0:1], 1.0)

        def quake_rsqrt(sl4):
            # rs = 1/sqrt(var+eps), all on DVE (avoids ACT Sqrt table swap)
            va = mv_all[:, sl4, 1]
            a = rs_all[:, sl4]
            nc.vector.tensor_scalar_add(a, va, float(EPS))
            u = a.bitcast(I32)
            y = q1[:, sl4]
            yi = y.bitcast(I32)
            nc.vector.tensor_scalar(
                yi, u, 1, 0, op0=OP.logical_shift_right, op1=OP.bypass
            )
            nc.vector.tensor_scalar(yi, yi, -1, QK3, op0=OP.mult, op1=OP.add)
            t = q2[:, sl4]
            for it in range(2):
                # y = y * (1.5 - 0.5*a*y*y)
                nc.vector.tensor_mul(t, y, y)
                nc.vector.tensor_mul(t, t, a)
                nc.vector.tensor_scalar(t, t, -0.5, 1.5, op0=OP.mult, op1=OP.add)
                if it == 0:
                    nc.vector.tensor_mul(y, y, t)
                else:
                    nc.vector.tensor_mul(rs_all[:, sl4], y, t)

        def prep_ln(b4):
            # DVE-only part of a prep block (no psum)
            for t in range(b4 * 4, b4 * 4 + 4):
                stats = sbTmp.tile([128, 6], F32, tag="st")
                nc.vector.bn_stats(stats, xkv_sb[:, t, :])
                nc.vector.bn_aggr(mv_all[:, t, :], stats)
            sl4 = slice(b4 * 4, b4 * 4 + 4)
            quake_rsqrt(sl4)
            for t in range(b4 * 4, b4 * 4 + 4):
                nc.vector.tensor_scalar(
                    xn0_sb[:, t, :],
                    xkv_sb[:, t, :],
                    mv_all[:, t, 0:1],
                    rs_all[:, t : t + 1],
                    op0=OP.subtract,
                    op1=OP.mult,
                )

        def prep_tp(b4):
            # 4 transposes into one sA tile, one ACT copy -> xkvT (bf16)
            tp = sA_tile("tp")
            for i, t in enumerate(range(b4 * 4, b4 * 4 + 4)):
                nc.tensor.transpose(
                    tp[:, i * 128 : (i + 1) * 128], xn0_sb[:, t, :], ident
                )
            nc.scalar.copy(
                xkvT[:, b4 * 512 : (b4 + 1) * 512], tp[:, 0:512]
            )

        def prep_kq(b4):
            c = b4
            pp = sA_tile("pp")
            nc.tensor.matmul(
                pp[:, 0:CHUNK], wk_f, xkvT[:, c * CHUNK : (c + 1) * CHUNK],
                start=True, stop=True,
            )
            nc.scalar.add(kT[:, c * CHUNK : (c + 1) * CHUNK], pp[:, 0:CHUNK], bke)
            if c < NCH:
                nc.tensor.matmul(
                    pp[:, CHUNK : 2 * CHUNK],
                    wq_f,
                    xkvT[:, c * CHUNK : (c + 1) * CHUNK],
                    start=True,
                    stop=True,
                )
                nc.scalar.add(
                    qT[:, c * CHUNK : (c + 1) * CHUNK], pp[:, CHUNK : 2 * CHUNK], bqe
                )

        def prep_v(b4):
            # 4 v-proj matmuls into one sA tile, one strided ACT copy
            vp = sA_tile("vp")
            for i, t in enumerate(range(b4 * 4, b4 * 4 + 4)):
                nc.tensor.matmul(
                    vp[:, i * 128 : (i + 1) * 128],
                    xkvT[:, t * 128 : (t + 1) * 128],
                    wv_f,
                    start=True,
                    stop=True,
                )
            sl4 = slice(b4 * 4, b4 * 4 + 4)
            vpv = vp[:, 0:512].rearrange("p (t h d) -> p t h d", t=4, h=4, d=32)
            nc.scalar.copy(v_sb[:, sl4, :, 1:33], vpv)

        # ---- attention ----
        pPool = tc.alloc_tile_pool(name="pPool", bufs=3)

        ctx_ps = {}

        def attn_scores(qc, kt):
            q0 = qc * CHUNK
            k0 = kt * 128
            # group A: heads 0,2 -> one [128,1024] psum tile, ACT exp
            sa = sA_tile("sa")
            for i, h in enumerate((0, 2)):
                nc.tensor.matmul(
                    sa[:, i * CHUNK : (i + 1) * CHUNK],
                    kT[h * DH : (h + 1) * DH, k0 : k0 + 128],
                    qT[h * DH : (h + 1) * DH, q0 : q0 + CHUNK],
                    start=True,
                    stop=True,
                    tile_position=(h * DH, 0),
                )
            pA = pPool.tile([128, 2 * CHUNK], BF16, tag="pA")
            nc.scalar.activation(pA, sa, AF.Exp, bias=nshift, scale=1.0)
            # group B: heads 1,3 -> per-head [128,512] psum tiles, DVE exp
            pBs = []
            for i, h in enumerate((1, 3)):
                sb = ps.tile(
                    [128, CHUNK], F32, name=f"sb{i}", tag=f"sB{i}", bufs=1
                )
                nc.tensor.matmul(
                    sb,
                    kT[h * DH : (h + 1) * DH, k0 : k0 + 128],
                    qT[h * DH : (h + 1) * DH, q0 : q0 + CHUNK],
                    start=True,
                    stop=True,
                    tile_position=(h * DH, 0),
                )
                pB = pPool.tile([128, CHUNK], I16, tag=f"pB{i}")
                nc.vector.tensor_scalar(pB, sb, SA, SB, op0=OP.mult, op1=OP.add)
                pBs.append(pB.bitcast(BF16))
            return pA, pBs

        def attn_ctx(kt, p_sb):
            pA, pBs = p_sb
            first, last = kt == 0, kt == NKT - 1
            # M=64 col-tiled: h at rows 64i..64i+31, row 64i+32 = den,
            # rows 64i+33..64i+63 = 0 (keeps the whole bank initialized)
            for i, h in enumerate((0, 2)):
                nc.tensor.matmul(
                    ctx_ps["A"][64 * i : 64 * i + 64, :],
                    v_sb[:, kt, h, :],
                    pA[:, i * CHUNK : (i + 1) * CHUNK],
                    start=first,
                    stop=last,
                    tile_position=(0, 64 * i),
                    skip_group_check=True,
                )
            for i, h in enumerate((1, 3)):
                nc.tensor.matmul(
                    ctx_ps["B"][64 * i : 64 * i + 64, :],
                    v_sb[:, kt, h, :],
                    pBs[i],
                    start=first,
                    stop=last,
                    tile_position=(0, 64 * i),
                    skip_group_check=True,
                )

        tail_state = {}

        def tail_copy(qc, cps):
            # psum->sbuf unload of both ctx banks (frees the banks)
            st = {}
            for g in ("A", "B"):
                cs = sbTmp.tile([128, CHUNK], F32, tag=f"cs{g}")
                nc.scalar.copy(cs, cps[g])
                st[g] = cs
            tail_state[qc] = st

        def tail_div(qc, g):
            # den broadcast via masked fp32 matmul, fast recip, multiply
            cs = tail_state[qc][g]
            dps = sA_tile(f"dps{g}")
            nc.tensor.matmul(dps[:, 0:CHUNK], msel, cs, start=True, stop=True)
            dinv = sbTmp.tile([128, CHUNK], F32, tag=f"di{g}")
            nc.vector.reciprocal_approx_fast(dinv, dps[:, 0:CHUNK])
            ctxn = sbTmp.tile([128, CHUNK], F32R, tag=f"cn{g}")
            nc.vector.tensor_mul(ctxn, cs, dinv)
            tail_state[qc][g + "n"] = ctxn

        def tail_out(qc):
            q0 = qc * CHUNK
            outp = sA_tile("outp")
            for gi, g in enumerate(("A", "B")):
                ctxn = tail_state[qc][g + "n"]
                nc.tensor.matmul(
                    outp[:, 0:CHUNK],
                    wo_sb[:, gi, :],
                    ctxn,
                    start=(gi == 0),
                    stop=(gi == 1),
                )
            fin = sbTmp.tile([128, CHUNK], F32, tag="fin")
            nc.vector.tensor_add(fin, outp[:, 0:CHUNK], residT[:, q0 : q0 + CHUNK])
            nc.sync.dma_start(out=outT_d[:, q0 : q0 + CHUNK], in_=fin)

        # ---- schedule ----
        prep_ln(0)
        prep_tp(0)
        prep_kq(0)
        prep_v(0)
        prep_ln(1)
        prep_tp(1)
        prep_kq(1)
        prep_v(1)
        # residT = xt + rbias (2x_2P SBUF mode)
        nc.vector.tensor_scalar_add(residT[:, 0:CHUNK], xt_sb[:, 0:CHUNK], rbias)
        nc.vector.tensor_scalar_add(
            residT[:, CHUNK:QH], xt_sb[:, CHUNK:QH], rbias
        )

        # chunk 0; prep blocks 2,3 dripped one psum tile per iteration
        ctx_ps = {
            "A": ps.tile([128, CHUNK], F32, name="ctxA0", tag="ctxA", bufs=1),
            "B": ps.tile([128, CHUNK], F32, name="ctxB0", tag="ctxB", bufs=1),
        }
        drip = [
            lambda: prep_ln(2),
            lambda: prep_tp(2),
            lambda: prep_kq(2),
            lambda: prep_v(2),
            lambda: prep_ln(3),
            lambda: prep_tp(3),
            lambda: prep_kq(3),
            lambda: prep_v(3),
        ]
        pending = attn_scores(0, 0)
        for kt in range(NKT):
            if kt >= 1 and drip:
                drip.pop(0)()
            nxt = attn_scores(0, kt + 1) if kt + 1 < NKT else None
            attn_ctx(kt, pending)
            pending = nxt

        ctx0 = ctx_ps
        tail_copy(0, ctx0)

        # chunk 1; chunk-0 tail pieces interleaved
        ctx_ps = {
            "A": ps.tile([128, CHUNK], F32, name="ctxA1", tag="ctxA", bufs=1),
            "B": ps.tile([128, CHUNK], F32, name="ctxB1", tag="ctxB", bufs=1),
        }
        pending = attn_scores(1, 0)
        for kt in range(NKT):
            if kt == 2:
                tail_div(0, "A")
            elif kt == 4:
                tail_div(0, "B")
            elif kt == 6:
                tail_out(0)
            nxt = attn_scores(1, kt + 1) if kt + 1 < NKT else None
            attn_ctx(kt, pending)
            pending = nxt

        # chunk-1 endgame: half-width pipeline so the first out-DMA starts early
        q0 = CHUNK
        csA = sbTmp.tile([128, CHUNK], F32, tag="csA")
        csB = sbTmp.tile([128, CHUNK], F32, tag="csB")
        diA = sbTmp.tile([128, CHUNK], F32, tag="diA")
        diB = sbTmp.tile([128, CHUNK], F32, tag="diB")
        cnA = sbTmp.tile([128, CHUNK], F32R, tag="cnA")
        cnB = sbTmp.tile([128, CHUNK], F32R, tag="cnB")
        fin = sbTmp.tile([128, CHUNK], F32, tag="fin")
        dps = sA_tile("dpsf")
        outp = sA_tile("outpf")
        for hf in range(2):
            sl = slice(hf * 256, (hf + 1) * 256)
            nc.scalar.copy(csA[:, sl], ctx_ps["A"][:, sl])
            nc.scalar.copy(csB[:, sl], ctx_ps["B"][:, sl])
            d0 = hf * 512
            nc.tensor.matmul(
                dps[:, d0 : d0 + 256], msel, csA[:, sl], start=True, stop=True
            )
            nc.tensor.matmul(
                dps[:, d0 + 256 : d0 + 512], msel, csB[:, sl], start=True, stop=True
            )
            nc.vector.reciprocal_approx_fast(diA[:, sl], dps[:, d0 : d0 + 256])
            nc.vector.reciprocal_approx_fast(diB[:, sl], dps[:, d0 + 256 : d0 + 512])
            nc.vector.tensor_mul(cnA[:, sl], csA[:, sl], diA[:, sl])
            nc.vector.tensor_mul(cnB[:, sl], csB[:, sl], diB[:, sl])
            o0 = hf * 256
            nc.tensor.matmul(
                outp[:, o0 : o0 + 256], wo_sb[:, 0, :], cnA[:, sl],
                start=True, stop=False,
            )
            nc.tensor.matmul(
                outp[:, o0 : o0 + 256], wo_sb[:, 1, :], cnB[:, sl],
                start=False, stop=True,
            )
            nc.vector.tensor_add(
                fin[:, sl], outp[:, o0 : o0 + 256], residT[:, q0 + hf * 256 : q0 + (hf + 1) * 256]
            )
            nc.sync.dma_start(
                out=outT_d[:, q0 + hf * 256 : q0 + (hf + 1) * 256], in_=fin[:, sl]
            )

        pPool.release()
        ps.release()
        sbTmp.release()
        sbBig.release()
        sbW.release()
        consts.release()

    nc.compile()
    return nc


def _get_compiled():
    global _compiled
    if _compiled is None:
        _compiled = _build()
    return _compiled


# device position j <- host row (j%128)*16 + j//128
_DEV2HOST = (np.arange(S) % 128) * NT + np.arange(S) // 128
_HOSTPERM = np.empty(S, dtype=np.int64)
_HOSTPERM[_DEV2HOST] = np.arange(S)


def kernel(x, Wq, bq, Wk, bk, Wv, bv, gamma, beta, Wo, bo):
    bf16 = mybir.dt.np(BF16)
    x = np.asarray(x, dtype=np.float32)
    Wq = np.asarray(Wq, dtype=np.float64)
    Wk = np.asarray(Wk, dtype=np.float64)
    Wv = np.asarray(Wv, dtype=np.float64)
    Wo = np.asarray(Wo, dtype=np.float64)
    gamma = np.asarray(gamma, dtype=np.float64)
    beta = np.asarray(beta, dtype=np.float64)
    bq = np.asarray(bq, dtype=np.float64)
    bk = np.asarray(bk, dtype=np.float64)
    bv = np.asarray(bv, dtype=np.float64)
    bo = np.asarray(bo, dtype=np.float64)

    # fold gamma (and ISQ into q) into the projections; beta into biases
    wq_f = np.ascontiguousarray((Wq * gamma[:, None] * ISQ).astype(bf16))
    wk_f = np.ascontiguousarray((Wk * gamma[:, None]).astype(bf16))
    wv_f = np.ascontiguousarray((Wv * gamma[:, None]).astype(bf16))
    bq_eff = (Wq.T @ beta + bq) * ISQ
    bk_eff = Wk.T @ beta + bk
    bv_eff = Wv.T @ beta + bv
    rbias = Wo.T @ bv_eff + bo

    # Wo rows permuted to the 2-bank ctx layout:
    # bank A holds h0 at partitions 0-31, h2 at 64-95; bank B h1/h3.
    woAB = np.zeros((2, D, D), dtype=np.float64)
    woAB[0, 1:33] = Wo[0 * DH : 1 * DH]
    woAB[0, 65:97] = Wo[2 * DH : 3 * DH]
    woAB[1, 1:33] = Wo[1 * DH : 2 * DH]
    woAB[1, 65:97] = Wo[3 * DH : 4 * DH]
    woAB = np.ascontiguousarray(woAB.astype(np.float32))

    vecs = np.ascontiguousarray(
        np.stack([bq_eff, bk_eff, rbias]).astype(np.float32)
    )

    nc = _get_compiled()

    in_maps = []
    for c in range(N_CORES):
        b, half = c // 2, c % 2
        off = half * QH
        xroll = np.roll(x[b], -off, axis=0)
        xin = np.ascontiguousarray(xroll[_HOSTPERM])
        xt = np.ascontiguousarray(xroll[0:QH].T)
        in_maps.append(
            {
                "xkv": xin,
                "xt": xt,
                "wq": wq_f,
                "wk": wk_f,
                "wv": wv_f,
                "woAB": woAB,
                "vecs": vecs,
            }
        )

    res = run_bass_kernel_spmd(nc, in_maps, core_ids=list(range(N_CORES)), trace=False)

    out = np.empty((B, S, D), dtype=np.float32)
    for c in range(N_CORES):
        b, half = c // 2, c % 2
        off = half * QH
        out[b, off : off + QH, :] = res.results[c]["outT"].T
    return out


# revision 18
# speedup vs baseline: 1.2919x; 1.0342x over previous
"""Multi-head self-attention (pre-LN, residual) Trainium2 Bass kernel, v2.

Problem: B=4, S=2048, D=128, H=4, Dh=32, fp32.
Sharding: 8 cores = 4 batches x 2 query-halves (1024 queries/core).
Each core receives its batch's full x, row-shuffled by the host so that
(a) the core's query half occupies device positions 0..1023 and (b) each
SBUF partition loads consecutive DRAM rows.

Dataflow ([feature, seq] layouts), per core:
  xn0^T --W--> Q^T,K^T [hd, s] bf16;  V [s, hd] bf16 with a per-head
                                      ones column appended (33 cols/head)
  S^T[k,q] = K^T.T @ Q^T   4 heads row-tiled (K=32 at rows h*32)
  P_A = exp(S^T - 8)       heads {0,2} on ACT (one [128,1024] op)
  P_B = schraudolph(S^T-8) heads {1,3} on DVE (per-head [128,512] ops:
                           int16(x*SA+SB) bits == bf16 exp)
  ctx+den fused: M=33 col-tiled matmuls, bankA={h0@0,h2@64},
                 bankB={h1@0,h3@64}; row 32/96 of each bank = den
  deninv: K=1 ones-matmul broadcasts den rows to [128,512], DVE fast
          reciprocal, multiply, then 4 K=32 row-positioned Wo matmuls
          accumulate the output projection (junk rows never read)
  out^T = Wo.T @ ctxn + (x^T + rbias)

Host folds gamma/beta/biases/ISQ into the projection weights (numpy),
permutes Wo rows to the 2-bank ctx layout, and ships x^T for the
residual, so the device does no weight prep.  LN rsqrt runs on DVE
(quake seed + 2 Newton steps) so ACT keeps one table set (exp) loaded.

PSUM (8 banks): sA scores [128,1024] x2 bufs = 4, sB0/sB1 per-head
[128,512] = 2, ctxA/ctxB = 2.  Prep and tail psum tiles ride the sA
ring (2-buf rotation absorbs single insertions); prep is dripped one
psum tile per attention iteration; chunk-0's tail overlaps chunk-1.
"""

import sys

if "/opt/trn_rl_repo" not in sys.path:
    sys.path.insert(0, "/opt/trn_rl_repo")

import numpy as np

import concourse.bacc as bacc
import concourse.tile as tile
import concourse.mybir as mybir
from concourse.bass_utils import run_bass_kernel_spmd
from concourse.masks import make_identity

F32 = mybir.dt.float32
F32R = mybir.dt.float32r
BF16 = mybir.dt.bfloat16
I16 = mybir.dt.int16
I32 = mybir.dt.int32
AF = mybir.ActivationFunctionType
OP = mybir.AluOpType

B, S, D = 4, 2048, 128
H, DH = 4, 32
N_CORES = 8
QH = S // 2  # queries per core
NT = S // 128  # 16 s-tiles
CHUNK = 512
NCH = QH // CHUNK  # q-chunks per core (2)
NKT = S // 128  # 16 k-tiles
EPS = 1e-6
SHIFT = 8.0
ISQ = 1.0 / np.sqrt(np.float32(DH))
# Schraudolph bf16 exp: int16(x*SA + SB).bits == bf16(exp(x - SHIFT))
SA = float(128.0 / np.log(2.0))
SB = float(127.0 * 128.0 - 0.0579 * 128.0 - SHIFT * 128.0 / np.log(2.0))
QK3 = 0x5F3759DF  # quake rsqrt seed

_compiled = None


def _build():
    nc = bacc.Bacc(
        "TRN2",
        target_bir_lowering=False,
        debug=False,
        enable_asserts=False,
        num_devices=N_CORES,
    )

    xkv_d = nc.dram_tensor("xkv", [S, D], F32, kind="ExternalInput").ap()
    xt_d = nc.dram_tensor("xt", [D, QH], F32, kind="ExternalInput").ap()
    wq_d = nc.dram_tensor("wq", [D, D], BF16, kind="ExternalInput").ap()
    wk_d = nc.dram_tensor("wk", [D, D], BF16, kind="ExternalInput").ap()
    wv_d = nc.dram_tensor("wv", [D, D], BF16, kind="ExternalInput").ap()
    # woAB[0] rows {0-31: h0, 64-95: h2}; woAB[1] rows {0-31: h1, 64-95: h3}
    woAB_d = nc.dram_tensor("woAB", [2, D, D], F32R, kind="ExternalInput").ap()
    # rows: bq_eff, bk_eff, rbias
    vecs_d = nc.dram_tensor("vecs", [3, D], F32, kind="ExternalInput").ap()
    outT_d = nc.dram_tensor("outT", [D, QH], F32, kind="ExternalOutput").ap()

    with tile.TileContext(nc) as tc:
        consts = tc.alloc_tile_pool(name="consts", bufs=1)
        sbW = tc.alloc_tile_pool(name="sbW", bufs=1)
        sbBig = tc.alloc_tile_pool(name="sbBig", bufs=1)
        sbTmp = tc.alloc_tile_pool(name="sbTmp", bufs=3)

        ident = consts.tile([128, 128], F32)
        make_identity(nc, ident)
        nshift = consts.tile([128, 1], F32)
        nc.vector.memset(nshift, -SHIFT)
        wsrc = consts.tile([128, 512], BF16)
        nc.vector.memset(wsrc, 0.5)
        wones = consts.tile([128, DH], BF16)
        nc.vector.memset(wones, 1.0)
        msel = consts.tile([128, 128], F32)
        nc.vector.memset(msel, 0.0)
        nc.vector.memset(msel[0:1, 0:64], 1.0)
        nc.vector.memset(msel[64:65, 64:128], 1.0)
        dummy = consts.tile([128, 1], F32)
        nc.vector.memset(dummy, 0.0)

        # ---- input DMAs ----
        wq_f = sbW.tile([D, D], BF16)
        wk_f = sbW.tile([D, D], BF16)
        wv_f = sbW.tile([D, D], BF16)
        wo_sb = sbW.tile([D, 2, D], F32R)
        nc.scalar.dma_start(out=wq_f, in_=wq_d)
        nc.scalar.dma_start(out=wk_f, in_=wk_d)
        nc.scalar.dma_start(out=wv_f, in_=wv_d)
        nc.scalar.dma_start(out=wo_sb, in_=woAB_d.rearrange("g d e -> d g e"))
        vecsT = sbW.tile([D, 3], F32)  # cols: bq_eff, bk_eff, rbias
        nc.scalar.dma_start(out=vecsT, in_=vecs_d.rearrange("v d -> d v"))
        residT = sbBig.tile([128, QH], F32)  # x^T + rbias (query half)
        xt_sb = sbBig.tile([128, QH], F32)

        xkv_sb = sbBig.tile([128, NT, 128], F32)
        xkv_r = xkv_d.rearrange("(p t) d -> p t d", t=NT)
        for c4 in range(4):
            nc.sync.dma_start(
                out=xkv_sb[:, c4 * 4 : (c4 + 1) * 4, :],
                in_=xkv_r[:, c4 * 4 : (c4 + 1) * 4, :],
            )
        nc.gpsimd.dma_start(out=xt_sb, in_=xt_d)

        # ---- PSUM pool: sA 2x[128,1024]=4 banks, sB0/sB1/ctxA/ctxB 1 each ----
        ps = tc.alloc_tile_pool(name="ps", bufs=1, space="PSUM")

        def sA_tile(name):
            return ps.tile([128, 2 * CHUNK], F32, name=name, tag="sA", bufs=2)

        # force the exp table load early (hides the ~1.3us load in startup)
        warm_exp = sbTmp.tile([128, 1], F32, tag="we")
        nc.scalar.activation(warm_exp, dummy, AF.Exp, bias=nshift, scale=1.0)

        # HAM warm-up chain on the ctx banks (no consumers -> back-to-back)
        for _ in range(8):
            wps = ps.tile([128, CHUNK], F32, name="wps", tag="ctxA", bufs=1)
            nc.tensor.matmul(wps[0:DH, :], wones, wsrc, start=True, stop=True)

        bqe = vecsT[:, 0:1]
        bke = vecsT[:, 1:2]
        rbias = vecsT[:, 2:3]


        # ---- LayerNorm + transposes + projections ----
        mv_all = sbBig.tile([128, NT, 2], F32)
        rs_all = sbBig.tile([128, NT], F32)
        nb_all = sbBig.tile([128, NT], F32)
        q1 = sbBig.tile([128, NT], F32)
        q2 = sbBig.tile([128, NT], F32)
        xn0_sb = sbBig.tile([128, NT, 128], F32)
        xkvT = sbBig.tile([128, S], BF16)  # xn0^T [d, s]
        kT = sbBig.tile([128, S], BF16)
        qT = sbBig.tile([128, QH], BF16)
        # V per head: 64 cols = [ones (den), 32 v-dims, 31 zeros]
        v_sb = sbBig.tile([128, NT, H, 64], BF16)
        nc.vector.memset(v_sb[:, :, :, 33:64], 0.0)
        nc.vector.memset(v_sb[:, :, :, 0:1], 1.0)

        def quake_rsqrt(sl4):
            # rs = 1/sqrt(var+eps), all on DVE (avoids ACT Sqrt table swap)
            va = mv_all[:, sl4, 1]
            a = rs_all[:, sl4]
            nc.vector.tensor_scalar_add(a, va, float(EPS))
            u = a.bitcast(I32)
            y = q1[:, sl4]
            yi = y.bitcast(I32)
            nc.vector.tensor_scalar(
                yi, u, 1, 0, op0=OP.logical_shift_right, op1=OP.bypass
            )
            nc.vector.tensor_scalar(yi, yi, -1, QK3, op0=OP.mult, op1=OP.add)
            t = q2[:, sl4]
            for it in range(2):
                # y = y * (1.5 - 0.5*a*y*y)
                nc.vector.tensor_mul(t, y, y)
                nc.vector.tensor_mul(t, t, a)
                nc.vector.tensor_scalar(t, t, -0.5, 1.5, op0=OP.mult, op1=OP.add)
                if it == 0:
                    nc.vector.tensor_mul(y, y, t)
                else:
                    nc.vector.tensor_mul(rs_all[:, sl4], y, t)

        def prep_ln(b4):
            # DVE-only part of a prep block (no psum)
            for t in range(b4 * 4, b4 * 4 + 4):
                stats = sbTmp.tile([128, 6], F32, tag="st")
                nc.vector.bn_stats(stats, xkv_sb[:, t, :])
                nc.vector.bn_aggr(mv_all[:, t, :], stats)
            sl4 = slice(b4 * 4, b4 * 4 + 4)
            quake_rsqrt(sl4)
            nc.vector.tensor_mul(nb_all[:, sl4], mv_all[:, sl4, 0], rs_all[:, sl4])
            nc.vector.tensor_scalar_mul(nb_all[:, sl4], nb_all[:, sl4], -1.0)

        def prep_tp(b4):
            # ACT: xn0 = x*rs - mu*rs, then 4 transposes into one sA tile,
            # one ACT copy -> xkvT (bf16)
            tp = sA_tile("tp")
            for i, t in enumerate(range(b4 * 4, b4 * 4 + 4)):
                nc.scalar.activation(
                    xn0_sb[:, t, :],
                    xkv_sb[:, t, :],
                    AF.Identity,
                    bias=nb_all[:, t : t + 1],
                    scale=rs_all[:, t : t + 1],
                )
                nc.tensor.transpose(
                    tp[:, i * 128 : (i + 1) * 128], xn0_sb[:, t, :], ident
                )
            nc.scalar.copy(
                xkvT[:, b4 * 512 : (b4 + 1) * 512], tp[:, 0:512]
            )

        def prep_kq(b4):
            c = b4
            pp = sA_tile("pp")
            nc.tensor.matmul(
                pp[:, 0:CHUNK], wk_f, xkvT[:, c * CHUNK : (c + 1) * CHUNK],
                start=True, stop=True,
            )
            nc.scalar.add(kT[:, c * CHUNK : (c + 1) * CHUNK], pp[:, 0:CHUNK], bke)
            if c < NCH:
                nc.tensor.matmul(
                    pp[:, CHUNK : 2 * CHUNK],
                    wq_f,
                    xkvT[:, c * CHUNK : (c + 1) * CHUNK],
                    start=True,
                    stop=True,
                )
                nc.scalar.add(
                    qT[:, c * CHUNK : (c + 1) * CHUNK], pp[:, CHUNK : 2 * CHUNK], bqe
                )

        def prep_v(b4):
            # 4 v-proj matmuls into one sA tile, one strided ACT copy
            vp = sA_tile("vp")
            for i, t in enumerate(range(b4 * 4, b4 * 4 + 4)):
                nc.tensor.matmul(
                    vp[:, i * 128 : (i + 1) * 128],
                    xkvT[:, t * 128 : (t + 1) * 128],
                    wv_f,
                    start=True,
                    stop=True,
                )
            sl4 = slice(b4 * 4, b4 * 4 + 4)
            vpv = vp[:, 0:512].rearrange("p (t h d) -> p t h d", t=4, h=4, d=32)
            nc.scalar.copy(v_sb[:, sl4, :, 1:33], vpv)

        # ---- attention ----
        pPool = tc.alloc_tile_pool(name="pPool", bufs=3)

        ctx_ps = {}

        def attn_scores(qc, kt):
            q0 = qc * CHUNK
            k0 = kt * 128
            # group A: heads 0,2 -> one [128,1024] psum tile, ACT exp
            sa = sA_tile("sa")
            for i, h in enumerate((0, 2)):
                nc.tensor.matmul(
                    sa[:, i * CHUNK : (i + 1) * CHUNK],
                    kT[h * DH : (h + 1) * DH, k0 : k0 + 128],
                    qT[h * DH : (h + 1) * DH, q0 : q0 + CHUNK],
                    start=True,
                    stop=True,
                    tile_position=(h * DH, 0),
                )
            pA = pPool.tile([128, 2 * CHUNK], BF16, tag="pA")
            nc.scalar.activation(pA, sa, AF.Exp, bias=nshift, scale=1.0)
            # group B: heads 1,3 -> per-head [128,512] psum tiles, DVE exp
            pBs = []
            for i, h in enumerate((1, 3)):
                sb = ps.tile(
                    [128, CHUNK], F32, name=f"sb{i}", tag=f"sB{i}", bufs=1
                )
                nc.tensor.matmul(
                    sb,
                    kT[h * DH : (h + 1) * DH, k0 : k0 + 128],
                    qT[h * DH : (h + 1) * DH, q0 : q0 + CHUNK],
                    start=True,
                    stop=True,
                    tile_position=(h * DH, 0),
                )
                pB = pPool.tile([128, CHUNK], I16, tag=f"pB{i}")
                nc.vector.tensor_scalar(pB, sb, SA, SB, op0=OP.mult, op1=OP.add)
                pBs.append(pB.bitcast(BF16))
            return pA, pBs

        def attn_ctx(kt, p_sb):
            pA, pBs = p_sb
            first, last = kt == 0, kt == NKT - 1
            # M=64 col-tiled: h at rows 64i..64i+31, row 64i+32 = den,
            # rows 64i+33..64i+63 = 0 (keeps the whole bank initialized)
            for i, h in enumerate((0, 2)):
                nc.tensor.matmul(
                    ctx_ps["A"][64 * i : 64 * i + 64, :],
                    v_sb[:, kt, h, :],
                    pA[:, i * CHUNK : (i + 1) * CHUNK],
                    start=first,
                    stop=last,
                    tile_position=(0, 64 * i),
                    skip_group_check=True,
                )
            for i, h in enumerate((1, 3)):
                nc.tensor.matmul(
                    ctx_ps["B"][64 * i : 64 * i + 64, :],
                    v_sb[:, kt, h, :],
                    pBs[i],
                    start=first,
                    stop=last,
                    tile_position=(0, 64 * i),
                    skip_group_check=True,
                )

        tail_state = {}

        def tail_copy(qc, cps):
            # psum->sbuf unload of both ctx banks (frees the banks)
            st = {}
            for g in ("A", "B"):
                cs = sbTmp.tile([128, CHUNK], F32, tag=f"cs{g}")
                nc.scalar.copy(cs, cps[g])
                st[g] = cs
            tail_state[qc] = st

        def tail_div(qc, g):
            # den broadcast via masked fp32 matmul, fast recip, multiply
            cs = tail_state[qc][g]
            dps = sA_tile(f"dps{g}")
            nc.tensor.matmul(dps[:, 0:CHUNK], msel, cs, start=True, stop=True)
            dinv = sbTmp.tile([128, CHUNK], F32, tag=f"di{g}")
            nc.vector.reciprocal_approx_fast(dinv, dps[:, 0:CHUNK])
            ctxn = sbTmp.tile([128, CHUNK], F32R, tag=f"cn{g}")
            nc.vector.tensor_mul(ctxn, cs, dinv)
            tail_state[qc][g + "n"] = ctxn

        def tail_out(qc):
            q0 = qc * CHUNK
            outp = sA_tile("outp")
            for gi, g in enumerate(("A", "B")):
                ctxn = tail_state[qc][g + "n"]
                nc.tensor.matmul(
                    outp[:, 0:CHUNK],
                    wo_sb[:, gi, :],
                    ctxn,
                    start=(gi == 0),
                    stop=(gi == 1),
                )
            fin = sbTmp.tile([128, CHUNK], F32, tag="fin")
            nc.vector.tensor_add(fin, outp[:, 0:CHUNK], residT[:, q0 : q0 + CHUNK])
            nc.sync.dma_start(out=outT_d[:, q0 : q0 + CHUNK], in_=fin)

        # ---- schedule ----
        prep_ln(0)
        prep_tp(0)
        prep_kq(0)

        def resid_adds():
            nc.vector.tensor_scalar_add(
                residT[:, 0:CHUNK], xt_sb[:, 0:CHUNK], rbias
            )
            nc.vector.tensor_scalar_add(
                residT[:, CHUNK:QH], xt_sb[:, CHUNK:QH], rbias
            )

        # chunk 0; prep blocks 2,3 dripped one psum tile per iteration
        ctx_ps = {
            "A": ps.tile([128, CHUNK], F32, name="ctxA0", tag="ctxA", bufs=1),
            "B": ps.tile([128, CHUNK], F32, name="ctxB0", tag="ctxB", bufs=1),
        }
        drip = [
            lambda: prep_v(0),
            lambda: prep_ln(1),
            lambda: prep_tp(1),
            lambda: prep_kq(1),
            lambda: prep_v(1),
            lambda: prep_ln(2),
            lambda: prep_tp(2),
            lambda: prep_kq(2),
            lambda: prep_v(2),
            lambda: prep_ln(3),
            lambda: prep_tp(3),
            lambda: prep_kq(3),
            lambda: prep_v(3),
            resid_adds,
        ]
        pending = attn_scores(0, 0)
        for kt in range(NKT):
            if drip:
                drip.pop(0)()
            nxt = attn_scores(0, kt + 1) if kt + 1 < NKT else None
            attn_ctx(kt, pending)
            pending = nxt

        ctx0 = ctx_ps
        tail_copy(0, ctx0)

        # chunk 1; chunk-0 tail pieces interleaved
        ctx_ps = {
            "A": ps.tile([128, CHUNK], F32, name="ctxA1", tag="ctxA", bufs=1),
            "B": ps.tile([128, CHUNK], F32, name="ctxB1", tag="ctxB", bufs=1),
        }
        def t0_dps(g):
            cs = tail_state[0][g]
            dps = sA_tile(f"dps{g}")
            nc.tensor.matmul(dps[:, 0:CHUNK], msel, cs, start=True, stop=True)
            tail_state[0][g + "d"] = dps

        def t0_recip(g):
            dinv = sbTmp.tile([128, CHUNK], F32, tag=f"di{g}")
            nc.vector.reciprocal_approx_fast(
                dinv, tail_state[0][g + "d"][:, 0:CHUNK]
            )
            tail_state[0][g + "i"] = dinv

        def t0_mult(g):
            ctxn = sbTmp.tile([128, CHUNK], F32R, tag=f"cn{g}")
            nc.vector.tensor_mul(ctxn, tail_state[0][g], tail_state[0][g + "i"])
            tail_state[0][g + "n"] = ctxn

        tail0 = {
            1: lambda: t0_dps("A"),
            2: lambda: t0_recip("A"),
            3: lambda: t0_mult("A"),
            5: lambda: t0_dps("B"),
            6: lambda: t0_recip("B"),
            7: lambda: t0_mult("B"),
            9: lambda: tail_out(0),
        }
        pending = attn_scores(1, 0)
        for kt in range(NKT):
            if kt in tail0:
                tail0[kt]()
            nxt = attn_scores(1, kt + 1) if kt + 1 < NKT else None
            attn_ctx(kt, pending)
            pending = nxt

        # chunk-1 endgame: half-width pipeline so the first out-DMA starts early
        q0 = CHUNK
        csA = sbTmp.tile([128, CHUNK], F32, tag="csA")
        csB = sbTmp.tile([128, CHUNK], F32, tag="csB")
        diA = sbTmp.tile([128, CHUNK], F32, tag="diA")
        diB = sbTmp.tile([128, CHUNK], F32, tag="diB")
        cnA = sbTmp.tile([128, CHUNK], F32R, tag="cnA")
        cnB = sbTmp.tile([128, CHUNK], F32R, tag="cnB")
        fin = sbTmp.tile([128, CHUNK], F32, tag="fin")
        dps = sA_tile("dpsf")
        outp = sA_tile("outpf")
        for hf in range(2):
            sl = slice(hf * 256, (hf + 1) * 256)
            nc.scalar.copy(csA[:, sl], ctx_ps["A"][:, sl])
            nc.scalar.copy(csB[:, sl], ctx_ps["B"][:, sl])
            d0 = hf * 512
            nc.tensor.matmul(
                dps[:, d0 : d0 + 256], msel, csA[:, sl], start=True, stop=True
            )
            nc.tensor.matmul(
                dps[:, d0 + 256 : d0 + 512], msel, csB[:, sl], start=True, stop=True
            )
            nc.vector.reciprocal_approx_fast(diA[:, sl], dps[:, d0 : d0 + 256])
            nc.vector.reciprocal_approx_fast(diB[:, sl], dps[:, d0 + 256 : d0 + 512])
            nc.vector.tensor_mul(cnA[:, sl], csA[:, sl], diA[:, sl])
            nc.vector.tensor_mul(cnB[:, sl], csB[:, sl], diB[:, sl])
            o0 = hf * 256
            nc.tensor.matmul(
                outp[:, o0 : o0 + 256], wo_sb[:, 0, :], cnA[:, sl],
                start=True, stop=False,
            )
            nc.tensor.matmul(
                outp[:, o0 : o0 + 256], wo_sb[:, 1, :], cnB[:, sl],
                start=False, stop=True,
            )
            nc.vector.tensor_add(
                fin[:, sl], outp[:, o0 : o0 + 256], residT[:, q0 + hf * 256 : q0 + (hf + 1) * 256]
            )
            nc.sync.dma_start(
                out=outT_d[:, q0 + hf * 256 : q0 + (hf + 1) * 256], in_=fin[:, sl]
            )

        pPool.release()
        ps.release()
        sbTmp.release()
        sbBig.release()
        sbW.release()
        consts.release()

    nc.compile()
    return nc


def _get_compiled():
    global _compiled
    if _compiled is None:
        _compiled = _build()
    return _compiled


# device position j <- host row (j%128)*16 + j//128
_DEV2HOST = (np.arange(S) % 128) * NT + np.arange(S) // 128
_HOSTPERM = np.empty(S, dtype=np.int64)
_HOSTPERM[_DEV2HOST] = np.arange(S)


def kernel(x, Wq, bq, Wk, bk, Wv, bv, gamma, beta, Wo, bo):
    bf16 = mybir.dt.np(BF16)
    x = np.asarray(x, dtype=np.float32)
    Wq = np.asarray(Wq, dtype=np.float64)
    Wk = np.asarray(Wk, dtype=np.float64)
    Wv = np.asarray(Wv, dtype=np.float64)
    Wo = np.asarray(Wo, dtype=np.float64)
    gamma = np.asarray(gamma, dtype=np.float64)
    beta = np.asarray(beta, dtype=np.float64)
    bq = np.asarray(bq, dtype=np.float64)
    bk = np.asarray(bk, dtype=np.float64)
    bv = np.asarray(bv, dtype=np.float64)
    bo = np.asarray(bo, dtype=np.float64)

    # fold gamma (and ISQ into q) into the projections; beta into biases
    wq_f = np.ascontiguousarray((Wq * gamma[:, None] * ISQ).astype(bf16))
    wk_f = np.ascontiguousarray((Wk * gamma[:, None]).astype(bf16))
    wv_f = np.ascontiguousarray((Wv * gamma[:, None]).astype(bf16))
    bq_eff = (Wq.T @ beta + bq) * ISQ
    bk_eff = Wk.T @ beta + bk
    bv_eff = Wv.T @ beta + bv
    rbias = Wo.T @ bv_eff + bo

    # Wo rows permuted to the 2-bank ctx layout:
    # bank A holds h0 at partitions 0-31, h2 at 64-95; bank B h1/h3.
    woAB = np.zeros((2, D, D), dtype=np.float64)
    woAB[0, 1:33] = Wo[0 * DH : 1 * DH]
    woAB[0, 65:97] = Wo[2 * DH : 3 * DH]
    woAB[1, 1:33] = Wo[1 * DH : 2 * DH]
    woAB[1, 65:97] = Wo[3 * DH : 4 * DH]
    woAB = np.ascontiguousarray(woAB.astype(np.float32))

    vecs = np.ascontiguousarray(
        np.stack([bq_eff, bk_eff, rbias]).astype(np.float32)
    )

    nc = _get_compiled()

    in_maps = []
    for c in range(N_CORES):
        b, half = c // 2, c % 2
        off = half * QH
        xroll = np.roll(x[b], -off, axis=0)
        xin = np.ascontiguousarray(xroll[_HOSTPERM])
        xt = np.ascontiguousarray(xroll[0:QH].T)
        in_maps.append(
            {
                "xkv": xin,
                "xt": xt,
                "wq": wq_f,
                "wk": wk_f,
                "wv": wv_f,
                "woAB": woAB,
                "vecs": vecs,
            }
        )

    res = run_bass_kernel_spmd(nc, in_maps, core_ids=list(range(N_CORES)), trace=False)

    out = np.empty((B, S, D), dtype=np.float32)
    for c in range(N_CORES):
        b, half = c // 2, c % 2
        off = half * QH
        out[b, off : off + QH, :] = res.results[c]["outT"].T
    return out
